# revision 22
# baseline (speedup 1.0000x reference)
"""BasisExpansionLayer Trainium2 kernel.

Full input x: [256, 512] f32. Full output: [256, 512 + 512*512 + 512] f32
laid out as [sin(x) | (x_i * x_j for the cartesian i,j grid) | x].

The pair block is symmetric (x_i*x_j == x_j*x_i), so the device computes
each unordered pair exactly once: for every i it emits the mod-512 band
j = i..i+256, i.e. band[b, i, t] = x[b,i] * x[b,(i+t)%512], t in [0,257).
That's 512*257 values instead of 512*512 (the t=256 band is covered from
both endpoints; 0.4% redundancy keeps every core's work identical).
Bands are stored as fp16 (products computed in fp32 from an exact fp32
scalar and an fp16 vector operand; max rel err ~7.5e-4, far inside the
2e-2 gate) which also unlocks the DVE 4x perf mode (2-byte packed
operands). Stores drop 4x vs the dense-f32 layout: 8.42MB/core vs
33.55MB, and the kernel runs at the per-core HBM store roofline.

Sharding: pure data parallel over the band index i — core c owns
i in [64c, 64c+64). Per-core inputs are data-rotated so one shared NEFF
serves all 8 cores:
  xw[b, u]  = fp16(x[b, (64c + u) % 512])   [256, 320]
  xs[b, k]  = x[b, 64c + k]                 [256, 64] f32
Per core:
  pair_out[b, k*257 + t] = xw[b, k+t] * xs[b, k]     (fp16, [256, 16448])
  sid_out[b, :64] = sin(xs[b, :]); sid_out[b, 64:] = xs[b, :]  (f32)
The host mirrors the bands into the full symmetric [512,512] grid during
the gather (pure layout: every value was computed on-device).
"""

import numpy as np

B = 256
D = 512
M = 8            # cores
IPC = D // M     # 64 i-values per core
W = D // 2 + 1   # 257: band width per i
PACK = IPC * W   # 16448 packed pair columns per core
XW = IPC + W - 1  # 320: rotated-window input width

_CACHE = {}

# sin(x) = y * p(y^2), y = x - round(x/2pi)*2pi (Cody-Waite), |y| <= pi.
# p coeffs: IRLS-minimax fit, end-to-end fp32 max abs err 5.3e-7.
SIN_COEFFS = [
    1.0,
    -0.166666641831398,
    0.00833331048488617,
    -0.0001984015543712303,
    2.752945647443994e-06,
    -2.467699466990325e-08,
    1.345159122978501e-10,
]
INV2PI = 0.15915494309189535
MAGIC = 12582912.0  # 1.5 * 2**23: fp32 round-to-nearest via add/sub
TWOPI_HI = 6.28125
TWOPI_LO = 0.0019353071795864769

# knobs: tile_plan = per-half store-tile sizes (i-counts, sum 64),
# act_every = every act_every-th multiply goes to the scalar engine
# (0 = all DVE; DVE fp16 runs 4x so ACT offload is off by default).
DEFAULT_CFG = dict(
    warm_plan=(4, 12),    # ramp store tiles for half 0 (own small pool)
    tile_plan=(16, 32),   # steady-state store tiles for half 0
    tile_plan2=(32, 32),  # half 1: pipeline already full, no ramp
    act_every=0,
    bufs=2,
    warm_bufs=2,
    repeat=1,
    sin_pos=2,       # emit the sin chain after this many pair tiles
    split_load=True,  # issue half-1 loads on the ACT HWDGE ring
    sin_from_sbuf=True,  # build the sin input by DVE copy, not DMA reload
    # software-pipelined x loads: the next repeat's loads issue after the
    # prefetch-th store tile, so the DMA queue never drains at a rep
    # boundary (inert at repeat=1).
    xbufs=2,
    prefetch=1,  # tile-1 boundary: de-conflicts with the sin chain +
    # sid store that fire at the sin_pos=2 boundary
    # merge_loads: host lays x out partition-major ([p, half*W+u]) so each
    # input loads with ONE full-tile DMA instead of two per-half DMAs.
    merge_loads=False,
    # every pair-store tile's columns split across both HWDGE rings
    # (sp/act run concurrently; one ring alone caps ~358 GB/s while the
    # per-core HBM port sustains ~390+).  sp gets the larger share since
    # it also carries the sid store, act the h1 loads.
    rings=("sp", "act"),
    ring_assign="csp",
    csp_w=(17, 15),
)


def _build_nc(cfg=None):
    import concourse.bass as bass  # noqa: F401
    import concourse.mybir as mybir
    import concourse.tile as tile
    from concourse import bacc

    cfg = {**DEFAULT_CFG, **(cfg or {})}
    warm_plan = tuple(cfg.get("warm_plan") or ())
    tile_plan = tuple(cfg["tile_plan"])
    tile_plan2 = tuple(cfg.get("tile_plan2") or ())
    act_every = cfg["act_every"]
    bufs = cfg["bufs"]
    warm_bufs = cfg.get("warm_bufs", 2)
    repeat = cfg["repeat"]
    sin_pos = cfg.get("sin_pos", 0)
    split_load = cfg.get("split_load", False)
    sin_from_sbuf = cfg.get("sin_from_sbuf", False)
    store_ring = cfg.get("store_ring", "sp")  # "sp" | "alt"
    # multi-queue store distribution (None => legacy store_ring behavior):
    #   rings: tuple of ring names for pair stores, from {"sp","act","pool"}
    #   ring_assign: "rr" (round-robin by tile) | "csp" (column split)
    rings = cfg.get("rings")
    ring_assign = cfg.get("ring_assign", "rr")
    load_eng = cfg.get("load_eng")  # None => legacy split_load | "sp"|"act"|"pool"
    sid_eng = cfg.get("sid_eng", "sp")
    xbufs = cfg.get("xbufs", 1)
    prefetch = cfg.get("prefetch")  # store-tile index at which to preload x
    merge_loads = cfg.get("merge_loads", False)
    csp_w = cfg.get("csp_w")
    sid16 = cfg.get("sid16", False)
    # no_xs: skip the f32 xs input; scalars/sin/identity come from a DVE
    # upconvert of the fp16 window's first 64 columns (slightly larger but
    # still in-gate error; saves the xs loads and their ring slots).
    no_xs = cfg.get("no_xs", False)
    # per-half store plans: (size, use_warm_pool) lists
    plan_h0 = [(g, True) for g in warm_plan] + [(g, False) for g in tile_plan]
    plan_h1 = (
        [(g, False) for g in tile_plan2] if tile_plan2 else list(plan_h0)
    )
    assert sum(g for g, _ in plan_h0) == IPC, plan_h0
    assert sum(g for g, _ in plan_h1) == IPC, plan_h1

    f32 = mybir.dt.float32
    f16 = mybir.dt.float16
    nc = bacc.Bacc("TRN2", target_bir_lowering=False, debug=False, num_devices=M)

    if merge_loads:
        # partition-major: row p holds both batch halves side by side
        xw = nc.dram_tensor("xw", [128, 2 * XW], f16, kind="ExternalInput")
        xs = (
            None
            if no_xs
            else nc.dram_tensor("xs", [128, 2 * IPC], f32, kind="ExternalInput")
        )
    else:
        xw = nc.dram_tensor("xw", [B, XW], f16, kind="ExternalInput")
        xs = (
            None
            if no_xs
            else nc.dram_tensor("xs", [B, IPC], f32, kind="ExternalInput")
        )
    # sin and identity merged into one [B, 2*IPC] tensor: per-partition DMA
    # runs of 512 B (the SDMA line-rate minimum) instead of 2x256 B RMW.
    sid_dt = f16 if sid16 else f32
    sid_out = nc.dram_tensor("sid_out", [B, 2 * IPC], sid_dt, kind="ExternalOutput")
    pair_out = nc.dram_tensor("pair_out", [B, PACK], f16, kind="ExternalOutput")

    def _eng(name):
        return {
            "sp": nc.sync,
            "act": nc.scalar,
            "pool": nc.gpsimd,
            "vec": nc.vector,
        }[name]

    with tile.TileContext(nc) as tc:
        with (
            tc.tile_pool(name="xp", bufs=xbufs) as xpool,
            tc.tile_pool(name="sp", bufs=1) as spool,
            tc.tile_pool(name="wp", bufs=warm_bufs) as wpool,
            tc.tile_pool(name="op", bufs=bufs) as opool,
        ):
            alu = mybir.AluOpType

            def emit_loads():
                if merge_loads:
                    e0 = _eng(load_eng) if load_eng is not None else nc.sync
                    e1 = (
                        _eng(load_eng)
                        if load_eng is not None
                        else (nc.scalar if split_load else nc.sync)
                    )
                    twb = xpool.tile([128, 2 * XW], f16, tag="xwall")
                    e1.dma_start(twb[:], xw[:, :])
                    xwt_ = [twb[:, 0:XW], twb[:, XW : 2 * XW]]
                    tsb = xpool.tile([128, 2 * IPC], f32, tag="xsall")
                    if no_xs:
                        for h in range(2):
                            nc.vector.tensor_copy(
                                tsb[:, h * IPC : (h + 1) * IPC],
                                twb[:, h * XW : h * XW + IPC],
                            )
                    else:
                        e0.dma_start(tsb[:], xs[:, :])
                    xst_ = [tsb[:, 0:IPC], tsb[:, IPC : 2 * IPC]]
                    return xwt_, xst_
                xwt_, xst_ = [], []
                for h in range(2):
                    rows = slice(h * 128, (h + 1) * 128)
                    # half-1 loads go out on the ACT HWDGE ring so both
                    # rings generate descriptors in parallel at t=0.
                    if load_eng is not None:
                        dma_eng = _eng(load_eng)
                    else:
                        dma_eng = nc.scalar if (split_load and h == 1) else nc.sync
                    t = xpool.tile([128, XW], f16, tag=f"xw{h}")
                    dma_eng.dma_start(t[:], xw[rows, :])
                    xwt_.append(t[:])
                    ts = xpool.tile([128, IPC], f32, tag=f"xs{h}")
                    if no_xs:
                        nc.vector.tensor_copy(ts[:], t[:, 0:IPC])
                    else:
                        dma_eng.dma_start(ts[:], xs[rows, :])
                    xst_.append(ts[:])
                return xwt_, xst_

            nxt = None
            for _rep in range(repeat):
                # software-pipelined x loads: with prefetch set, rep k+1's
                # loads were already issued mid-way through rep k's store
                # stream, so the rep boundary never drains the DMA queue.
                xwt, xst = emit_loads() if nxt is None else nxt
                nxt = None

                def emit_sin():
                    # accurate sin via odd polynomial on DVE: both batch
                    # halves side by side in one [128, 2*IPC] tile.
                    Ws = 2 * IPC
                    xsin = spool.tile([128, Ws], f32, tag="xsin")
                    for h in range(2):
                        if sin_from_sbuf:
                            nc.vector.tensor_copy(
                                xsin[:, h * IPC : (h + 1) * IPC], xst[h][:]
                            )
                        else:
                            nc.sync.dma_start(
                                xsin[:, h * IPC : (h + 1) * IPC],
                                xs[h * 128 : (h + 1) * 128, :],
                            )
                    tt = spool.tile([128, Ws], f32, tag="t")
                    # t = x*inv2pi + magic ; k = t - magic (round-to-nearest)
                    nc.vector.tensor_scalar(
                        out=tt[:], in0=xsin[:], scalar1=INV2PI, scalar2=MAGIC,
                        op0=alu.mult, op1=alu.add,
                    )
                    kk = spool.tile([128, Ws], f32, tag="k")
                    nc.vector.tensor_scalar_sub(kk[:], tt[:], MAGIC)
                    # y = (x - k*2pi_hi) - k*2pi_lo
                    kh = spool.tile([128, Ws], f32, tag="kh")
                    nc.vector.tensor_scalar_mul(kh[:], kk[:], TWOPI_HI)
                    yy = spool.tile([128, Ws], f32, tag="y")
                    nc.vector.tensor_sub(yy[:], xsin[:], kh[:])
                    kl = spool.tile([128, Ws], f32, tag="kl")
                    nc.vector.tensor_scalar_mul(kl[:], kk[:], TWOPI_LO)
                    nc.vector.tensor_sub(yy[:], yy[:], kl[:])
                    uu = spool.tile([128, Ws], f32, tag="u")
                    nc.vector.tensor_mul(uu[:], yy[:], yy[:])
                    # Horner: p = (((c6*u + c5)*u + c4)...)*u + c0
                    pp = spool.tile([128, Ws], f32, tag="p")
                    nc.vector.tensor_scalar(
                        out=pp[:], in0=uu[:], scalar1=SIN_COEFFS[6],
                        scalar2=SIN_COEFFS[5], op0=alu.mult, op1=alu.add,
                    )
                    for cidx in (4, 3, 2, 1, 0):
                        nc.vector.tensor_mul(pp[:], pp[:], uu[:])
                        nc.vector.tensor_scalar_add(
                            pp[:], pp[:], SIN_COEFFS[cidx]
                        )
                    for h in range(2):
                        rows = slice(h * 128, (h + 1) * 128)
                        sid = spool.tile([128, Ws], sid_dt, tag=f"sid{h}")
                        sl = slice(h * IPC, (h + 1) * IPC)
                        # final Horner multiply lands directly in the merged
                        # tile; identity columns are a DVE copy of xs.
                        nc.vector.tensor_mul(sid[:, 0:IPC], pp[:, sl], yy[:, sl])
                        nc.vector.tensor_copy(sid[:, IPC : 2 * IPC], xst[h][:])
                        _eng(sid_eng).dma_start(sid_out[rows, :], sid[:])

                if sin_pos == 0:
                    emit_sin()

                # pair bands: out[p, k*257 + t] = xw[p, k+t] * xs[p, k]
                n_op = 0
                n_tile = 0
                for h in range(2):
                    rows = slice(h * 128, (h + 1) * 128)
                    i0 = 0
                    for g_sz, warm in (plan_h0 if h == 0 else plan_h1):
                        pool = wpool if warm else opool
                        ot = pool.tile(
                            [128, g_sz * W], f16, tag="warm" if warm else "out"
                        )
                        for k in range(g_sz):
                            i = i0 + k
                            dst = ot[:, k * W : (k + 1) * W]
                            src = xwt[h][:, i : i + W]
                            scal = xst[h][:, i : i + 1]
                            if act_every and n_op % act_every == act_every - 1:
                                # ACT: out = in * scale (activation Copy)
                                nc.scalar.mul(dst, src, scal)
                            else:
                                nc.vector.tensor_scalar_mul(dst, src, scal)
                            n_op += 1
                        if rings is None:
                            st_eng = (
                                nc.scalar
                                if (store_ring == "alt" and n_tile % 2 == 1)
                                else nc.sync
                            )
                            st_eng.dma_start(
                                pair_out[rows, i0 * W : (i0 + g_sz) * W], ot[:]
                            )
                        elif ring_assign == "csp":
                            # split this tile's columns across all rings
                            nr = len(rings)
                            if csp_w:
                                tot = sum(csp_w)
                                gcs = [g_sz * w // tot for w in csp_w]
                                for r in range(g_sz - sum(gcs)):
                                    gcs[r % nr] += 1
                            else:
                                gcs = [
                                    g_sz // nr + (1 if r < g_sz % nr else 0)
                                    for r in range(nr)
                                ]
                            c0 = 0
                            for r in range(nr):
                                gc = gcs[r]
                                if gc == 0:
                                    continue
                                _eng(rings[r]).dma_start(
                                    pair_out[
                                        rows, (i0 + c0) * W : (i0 + c0 + gc) * W
                                    ],
                                    ot[:, c0 * W : (c0 + gc) * W],
                                )
                                c0 += gc
                        else:  # "rr"
                            _eng(rings[n_tile % len(rings)]).dma_start(
                                pair_out[rows, i0 * W : (i0 + g_sz) * W], ot[:]
                            )
                        i0 += g_sz
                        n_tile += 1
                        if n_tile == sin_pos:
                            emit_sin()
                        if (
                            prefetch is not None
                            and n_tile == prefetch
                            and _rep + 1 < repeat
                        ):
                            nxt = emit_loads()
    nc.compile()
    return nc


def _get_nc(cfg=None):
    key = repr(cfg)
    if key not in _CACHE:
        _CACHE[key] = _build_nc(cfg)
    return _CACHE[key]


def _in_maps(x, cfg=None):
    eff = {**DEFAULT_CFG, **(cfg or {})}
    merged = eff.get("merge_loads", False)
    no_xs = eff.get("no_xs", False)
    x = np.ascontiguousarray(np.asarray(x, dtype=np.float32))
    assert x.shape == (B, D)
    # doubled x for wrap-free rotated windows
    x2 = np.concatenate([x, x[:, : XW]], axis=1)
    x2_f16 = x2.astype(np.float16)

    def _pm(a):
        # partition-major: row p holds both batch halves side by side
        w = a.shape[1]
        return np.ascontiguousarray(
            a.reshape(2, 128, w).transpose(1, 0, 2).reshape(128, 2 * w)
        )

    maps = []
    for c in range(M):
        xw_c = x2_f16[:, c * IPC : c * IPC + XW]
        xs_c = x[:, c * IPC : (c + 1) * IPC]
        m = {"xw": _pm(xw_c) if merged else np.ascontiguousarray(xw_c)}
        if not no_xs:
            m["xs"] = _pm(xs_c) if merged else np.ascontiguousarray(xs_c)
        maps.append(m)
    return maps


def _get_exec(cfg=None):
    """Build the 8-core sharded PJRT callable once per process.

    Mirrors bass2jax.run_bass_via_pjrt's multi-core path, but caches the
    jitted executable: loading/executing a second NEFF in the same process
    can wedge the exec unit, while re-executing one cached executable with
    donated output buffers is reliable.
    """
    key = ("exec", repr(cfg))
    if key in _CACHE:
        return _CACHE[key]

    import jax
    from jax.sharding import Mesh, PartitionSpec
    from jax.experimental.shard_map import shard_map
    import concourse.mybir as mybir
    from concourse import bass2jax

    nc = _get_nc(cfg)
    bass2jax.install_neuronx_cc_hook()

    partition_name = nc.partition_id_tensor.name if nc.partition_id_tensor else None
    in_names, out_names, out_avals, out_shapes = [], [], [], []
    for alloc in nc.m.functions[0].allocations:
        if not isinstance(alloc, mybir.MemoryLocationSet):
            continue
        name = alloc.memorylocations[0].name
        if alloc.kind == "ExternalInput":
            if name != partition_name:
                in_names.append(name)
        elif alloc.kind == "ExternalOutput":
            shape = tuple(alloc.tensor_shape)
            dtype = mybir.dt.np(alloc.dtype)
            out_names.append(name)
            out_avals.append(jax.core.ShapedArray(shape, dtype))
            out_shapes.append((shape, dtype))
    n_params = len(in_names)
    n_outs = len(out_avals)
    all_in_names = list(in_names) + list(out_names)
    if partition_name is not None:
        all_in_names.append(partition_name)

    def _body(*args):
        operands = list(args)
        if partition_name is not None:
            operands.append(bass2jax.partition_id_tensor())
        return tuple(
            bass2jax._bass_exec_p.bind(
                *operands,
                out_avals=tuple(out_avals),
                in_names=tuple(all_in_names),
                out_names=tuple(out_names),
                lowering_input_output_aliases=(),
                sim_require_finite=True,
                sim_require_nnan=True,
                nc=nc,
            )
        )

    devices = jax.devices()[:M]
    assert len(devices) == M, f"need {M} NeuronCores, found {len(devices)}"
    mesh = Mesh(np.asarray(devices), ("core",))
    in_specs = (PartitionSpec("core"),) * (n_params + n_outs)
    out_specs = (PartitionSpec("core"),) * n_outs
    donate = tuple(range(n_params, n_params + n_outs))
    sharded = jax.jit(
        shard_map(_body, mesh=mesh, in_specs=in_specs, out_specs=out_specs,
                  check_rep=False),
        donate_argnums=donate,
        keep_unused=True,
    )

    def run(in_maps):
        concat_in = [
            np.concatenate([np.asarray(in_maps[c][n]) for c in range(M)], axis=0)
            for n in in_names
        ]
        concat_zeros = [
            np.zeros((M * s[0], *s[1:]), dt) for s, dt in out_shapes
        ]
        outs = sharded(*concat_in, *concat_zeros)
        return [
            {
                name: np.asarray(outs[i]).reshape(M, *out_shapes[i][0])[c]
                for i, name in enumerate(out_names)
            }
            for c in range(M)
        ]

    _CACHE[key] = run
    return run


def _run(x, cfg=None):
    from concourse._compat import axon_active

    if axon_active():
        return _get_exec(cfg)(_in_maps(x, cfg))
    # native NRT path (no axon): run_bass_kernel_spmd handles the NEFF
    # load/exec/unload lifecycle per call.
    from concourse import bass_utils

    res = bass_utils.run_bass_kernel_spmd(
        _get_nc(cfg), _in_maps(x, cfg), core_ids=list(range(M))
    )
    return res.results


def _unpack_pair(results, out_pair):
    """Mirror the device-computed bands into the full [B, D, D] grid.

    P[b, i, t] = x_i * x_{(i+t)%512}, t in [0, 257).  Row i of the full
    grid in "unwrapped" coordinates FF[b, i, i:i+512] is [band_i |
    reversed anti-diagonal of P] and the final output folds FF's columns
    mod 512.  All steps are bulk numpy copies; the only non-contiguous
    reads are the inherent band->row transpose (anti-diagonal gather).
    """
    # doubled-i band array (wrap-free anti-diagonals), f32
    P2 = np.empty((B, 2 * D, W), np.float32)
    for c in range(M):
        P2[:, c * IPC : (c + 1) * IPC, :] = (
            results[c]["pair_out"].reshape(B, IPC, W).astype(np.float32)
        )
    P2[:, D:, :] = P2[:, :D, :]
    q0, q1, q2 = P2.strides

    # zero-filled so the mod-512 column fold is a single vectorized add:
    # row i only writes unwrapped cols [i, i+512), the rest stays 0.
    FF = np.zeros((B, D, 2 * D), np.float32)
    s0, s1, s2 = FF.strides
    # FF[b, i, i+t] = P2[b, i, t] for t in [0, 257): sliding-window view
    ff_main = np.lib.stride_tricks.as_strided(
        FF, shape=(B, D, W), strides=(s0, s1 + s2, s2)
    )
    ff_main[:] = P2[:, :D, :]
    # FF[b, i, i+257+k] = P2[b, i+257+k, 255-k] for k in [0, 255):
    # the value x_i*x_{(i+257+k)%512} lives in the band of its OTHER
    # endpoint j=i+257+k at offset 512-(257+k) = 255-k (anti-diagonal).
    ff_mir = np.lib.stride_tricks.as_strided(
        FF[:, :, W:], shape=(B, D, D - W), strides=(s0, s1 + s2, s2)
    )
    ad_origin = P2[:, W:, W - 2 :]  # data pointer at (0, 257, 255)
    ff_mir[:] = np.lib.stride_tricks.as_strided(
        ad_origin, shape=(B, D, D - W), strides=(q0, q1, q1 - q2)
    )
    # fold unwrapped columns mod 512: exactly one of the two halves is
    # nonzero at every position
    np.add(FF[:, :, :D], FF[:, :, D:], out=out_pair)


def kernel(**inputs):
    results = _run(inputs["x"])
    out = np.empty((B, 2 * D + D * D), dtype=np.float32)
    for c in range(M):
        r = results[c]
        out[:, c * IPC : (c + 1) * IPC] = r["sid_out"][:, :IPC]
        out[:, D + D * D + c * IPC : D + D * D + (c + 1) * IPC] = r["sid_out"][:, IPC:]
    _unpack_pair(results, out[:, D : D + D * D].reshape(B, D, D))
    return out
